# revision 1
# baseline (speedup 1.0000x reference)
"""TRN2 Bass kernel for nn_CausalSelfAttention_4054449128214.

The reference returns out_s + stop_gradient(out_full - out_s), whose forward
value is exactly out_full — plain dense causal self-attention. So the kernel
computes: qkv = x@W_attn+b_attn, per-head causal softmax attention, y@W_proj+b_proj.

Sharding (8 cores, no collectives):
  Megatron head-parallel. Cores 0-3 own head pairs (0,1)..(6,7); cores 4-7 own
  heads 8..11 (run twice for SPMD shape-uniformity, second copy's W_proj rows
  zeroed). Each core computes its heads' QKV columns, attention, and a partial
  row-sliced output projection; the host sums the 8 partials (the Megatron
  row-parallel all-reduce) and transposes back.

All matmuls run as float32r (TF32-class, ~13-bit mantissa, full PE rate at
free-dim >= 256); accumulation is exact fp32 in PSUM.
"""

import numpy as np

import concourse.bacc as bacc
import concourse.mybir as mybir
import concourse.tile as tile
from concourse.bass_utils import run_bass_kernel_spmd

F32 = mybir.dt.float32
F32R = mybir.dt.float32r

T = 1024          # sequence length
C = 768           # channels
NH = 12           # heads
HS = 64           # head size
NCORES = 8
TT = 512          # t-tile (matmul moving free dim)
NT = T // TT      # 2
NCC = C // 128    # 6 contraction chunks
NKC = T // 128    # 8 key chunks
SCALE = 1.0 / 8.0  # 1/sqrt(HS)

# core -> (head0, head1); cores 4-7 duplicate their head (2nd W_proj slice zeroed)
HEAD_MAP = [(0, 1), (2, 3), (4, 5), (6, 7), (8, 8), (9, 9), (10, 10), (11, 11)]

_CACHE: dict = {}


def _build_program():
    nc = bacc.Bacc("TRN2", target_bir_lowering=False, debug=False,
                   num_devices=NCORES)
    xT = nc.dram_tensor("xT", [C, T], F32, kind="ExternalInput").ap()
    wsel = nc.dram_tensor("wsel", [C, 384], F32, kind="ExternalInput").ap()
    wp = nc.dram_tensor("wp", [128, C], F32, kind="ExternalInput").ap()
    bqk = nc.dram_tensor("bqk", [128, 3], F32, kind="ExternalInput").ap()
    bpr = nc.dram_tensor("bpr", [128, NCC], F32, kind="ExternalInput").ap()
    eye2 = nc.dram_tensor("eye2", [128, HS], F32, kind="ExternalInput").ap()
    ones = nc.dram_tensor("ones", [128, 130], F32, kind="ExternalInput").ap()
    outT = nc.dram_tensor("outT", [C, T], F32, kind="ExternalOutput").ap()

    with tile.TileContext(nc) as tc:
        with (
            tc.tile_pool(name="const", bufs=1) as cp,
            tc.tile_pool(name="e", bufs=8) as ep,
            tc.tile_pool(name="rb", bufs=4) as rbp,
            tc.tile_pool(name="pmm", bufs=2, space="PSUM") as pmm,
            tc.tile_pool(name="pst", bufs=3, space="PSUM") as pst,
            tc.tile_pool(name="pov", bufs=2, space="PSUM") as pov,
            tc.tile_pool(name="ptr", bufs=1, space="PSUM") as ptr,
        ):
            # ---- big merged loads on SP; small constants on Pool's queue ----
            wt = cp.tile([128, NCC * 384], F32R, tag="wt")
            xt0 = cp.tile([128, NCC * TT], F32R, tag="xt0")
            xt1 = cp.tile([128, NCC * TT], F32R, tag="xt1")
            wsel3 = wsel.rearrange("(c p) j -> p c j", p=128).bitcast(F32R)
            x03 = xT[:, 0:TT].rearrange("(c p) t -> p c t", p=128).bitcast(F32R)
            x13 = xT[:, TT:T].rearrange("(c p) t -> p c t", p=128).bitcast(F32R)
            # interleave thirds so chunk cc lands early
            for c0, c1 in ((0, 2), (2, 4), (4, 6)):
                nc.sync.dma_start(
                    out=wt[:].rearrange("p (c j) -> p c j", c=NCC)[:, c0:c1],
                    in_=wsel3[:, c0:c1])
                nc.sync.dma_start(
                    out=xt0[:].rearrange("p (c t) -> p c t", c=NCC)[:, c0:c1],
                    in_=x03[:, c0:c1])
            for pc in range(3):
                c0, c1 = pc * 2, pc * 2 + 2
                nc.sync.dma_start(
                    out=xt1[:].rearrange("p (c t) -> p c t", c=NCC)[:, c0:c1],
                    in_=x13[:, c0:c1])
            wpt = cp.tile([64, 2 * C], F32R, tag="wpt")
            nc.sync.dma_start(
                out=wpt[:].rearrange("p (h e) -> p h e", h=2),
                in_=wp.rearrange("(h p) e -> p h e", p=64).bitcast(F32R))
            xts = [[xt0[:, cc * TT:(cc + 1) * TT], xt1[:, cc * TT:(cc + 1) * TT]]
                   for cc in range(NCC)]
            ws = [wt[:, cc * 384:(cc + 1) * 384] for cc in range(NCC)]
            wps = [wpt[:, hi * C:(hi + 1) * C] for hi in range(2)]

            bqk_sb = cp.tile([128, 3], F32, tag="bqk")
            nc.gpsimd.dma_start(out=bqk_sb[:], in_=bqk)
            eye_sb = cp.tile([128, HS], F32R, tag="eye")
            nc.gpsimd.dma_start(out=eye_sb[:], in_=eye2.bitcast(F32R))
            ones_sb = cp.tile([128, 2], F32R, tag="ones_sb")
            nc.gpsimd.dma_start(out=ones_sb[:], in_=ones[:, 0:2].bitcast(F32R))
            vaug = [cp.tile([128, 130], F32R, tag=f"va{kc}", name=f"va{kc}")
                    for kc in range(NKC)]
            for kc in range(NKC):
                # ones columns at 64 and 129 (cols 0:64 / 65:129 overwritten later)
                nc.vector.tensor_copy(
                    vaug[kc][:, 64:130:65], ones_sb[:])
            bpr_sb = cp.tile([128, NCC], F32, tag="bpr")
            nc.gpsimd.dma_start(out=bpr_sb[:], in_=bpr)

            # static causal masks for the DVE half of the mask work
            masks = []
            for kcr in range(4):
                m = cp.tile([128, TT], F32, tag=f"mask{kcr}", name=f"mask{kcr}")
                nc.vector.memset(m[:], 1.0)
                nc.gpsimd.affine_select(
                    m[:], m[:], pattern=[[1, TT]],
                    compare_op=mybir.AluOpType.is_ge, fill=0.0,
                    base=-128 * kcr, channel_multiplier=-1)
                masks.append(m)

            qkvT = [[None] * NT for _ in range(3)]
            yT = [[None] * NT for _ in range(2)]
            ost = [cp.tile([128, 2 * TT], F32, tag=f"ost{tt}{h}", name=f"ost{tt}{h}")
                   for tt in range(NT) for h in range(3)]

            def emit_qkv(tt):
                for mt in (2, 0, 1):
                    qkvT[mt][tt] = cp.tile([128, TT], F32R, tag=f"qkv{mt}_{tt}",
                                           name=f"qkv{mt}_{tt}")
                    ps = pmm.tile([128, TT], F32, tag="mm")
                    for cc in range(NCC):
                        nc.tensor.matmul(
                            ps[:], ws[cc][:, mt * 128:(mt + 1) * 128],
                            xts[cc][tt], start=(cc == 0), stop=(cc == NCC - 1))
                    nc.vector.tensor_scalar_add(
                        qkvT[mt][tt][:], ps[:], bqk_sb[:, mt:mt + 1])

            def emit_vaug(tt):
                for kc in range(tt * 4, tt * 4 + 4):
                    col = (kc % 4) * 128
                    for hi in range(2):
                        pt = ptr.tile([128, HS], F32R, tag="pt")
                        nc.tensor.transpose(
                            pt[:], qkvT[2][tt][hi * 64:(hi + 1) * 64, col:col + 128],
                            eye_sb[hi * 64:(hi + 1) * 64, :])
                        nc.vector.tensor_copy(vaug[kc][:, hi * 65:hi * 65 + 64], pt[:])

            def emit_attn(qt):
                for hi in range(2):
                    nlive = qt * 4 + 4
                    po = pov.tile([65, TT], F32, tag="po")
                    for kc in range(nlive):
                        ktile = qkvT[1][kc // 4]
                        kcol = (kc % 4) * 128
                        ps = pst.tile([128, TT], F32, tag="st")
                        nc.tensor.matmul(
                            ps[:], ktile[hi * 64:(hi + 1) * 64, kcol:kcol + 128],
                            qkvT[0][qt][hi * 64:(hi + 1) * 64, :],
                            start=True, stop=True)
                        e = ep.tile([128, TT], F32R, tag="e")
                        nc.scalar.activation(
                            e[:], ps[:], mybir.ActivationFunctionType.Exp,
                            scale=SCALE)
                        kcr = kc - qt * 4
                        if kcr >= 0:  # diagonal chunk: zero where tk > tq
                            if kcr % 2 == 0:
                                nc.gpsimd.affine_select(
                                    e[:], e[:], pattern=[[1, TT]],
                                    compare_op=mybir.AluOpType.is_ge, fill=0.0,
                                    base=-128 * kcr, channel_multiplier=-1)
                            else:
                                nc.vector.tensor_mul(e[:], e[:], masks[kcr][:])
                        nc.tensor.matmul(
                            po[:], vaug[kc][:, hi * 65:(hi + 1) * 65], e[:],
                            start=(kc == 0), stop=(kc == nlive - 1))
                    rb = rbp.tile([128, TT], F32, tag="rb")
                    nc.vector.reciprocal(rb[0:1, :], po[64:65, :])
                    rbc = rbp.tile([64, TT], F32, tag="rbc")
                    nc.gpsimd.partition_broadcast(rbc[:], rb[0:1, :])
                    yT[hi][qt] = cp.tile([64, TT], F32R, tag=f"y{hi}_{qt}",
                                         name=f"y{hi}_{qt}")
                    nc.vector.tensor_mul(yT[hi][qt][:], po[0:64, :], rbc[:])

            def emit_proj(tt):
                for half in range(3):
                    stile = ost[tt * 3 + half]
                    for ei in range(2):
                        et = half * 2 + ei
                        pm = pmm.tile([128, TT], F32, tag="mm")
                        for hi in range(2):
                            nc.tensor.matmul(
                                pm[:], wps[hi][:, et * 128:(et + 1) * 128],
                                yT[hi][tt][:], start=(hi == 0), stop=(hi == 1))
                        dst = stile[:, ei * TT:(ei + 1) * TT]
                        if et % 2 == 0:
                            nc.scalar.activation(
                                dst, pm[:], mybir.ActivationFunctionType.Identity,
                                bias=bpr_sb[:, et:et + 1])
                        else:
                            nc.vector.tensor_scalar_add(dst, pm[:], bpr_sb[:, et:et + 1])
                    nc.sync.dma_start(
                        out=outT[half * 256:(half + 1) * 256, tt * TT:(tt + 1) * TT]
                        .rearrange("(g p) t -> p g t", p=128),
                        in_=stile[:].rearrange("p (g t) -> p g t", g=2))

            emit_qkv(0)
            emit_vaug(0)
            emit_attn(0)
            emit_qkv(1)
            emit_vaug(1)
            emit_proj(0)
            emit_attn(1)
            emit_proj(1)
    nc.compile()
    return nc


def _in_maps(x, W_attn, b_attn, W_proj, b_proj):
    xTn = np.ascontiguousarray(x.reshape(T, C).T)  # [C, T]
    eye2 = np.ascontiguousarray(np.tile(np.eye(HS, dtype=np.float32), (2, 1)))
    maps = []
    for core in range(NCORES):
        h0, h1 = HEAD_MAP[core]
        cols = []
        for part in range(3):  # q, k, v column groups of W_attn
            for h in (h0, h1):
                cols.extend(range(part * C + h * HS, part * C + (h + 1) * HS))
        wsel = np.ascontiguousarray(W_attn[:, cols])                    # [C, 384]
        bqk = np.stack(
            [np.concatenate([b_attn[p * C + h0 * HS:p * C + (h0 + 1) * HS],
                             b_attn[p * C + h1 * HS:p * C + (h1 + 1) * HS]])
             for p in range(3)], axis=1).astype(np.float32)             # [128, 3]
        wpc = np.concatenate(
            [W_proj[h0 * HS:(h0 + 1) * HS, :],
             np.zeros_like(W_proj[:HS]) if h1 == h0
             else W_proj[h1 * HS:(h1 + 1) * HS, :]], axis=0)            # [128, C]
        bpr = (b_proj.reshape(NCC, 128).T if core == 0
               else np.zeros((128, NCC), np.float32)).astype(np.float32)
        maps.append({
            "xT": xTn, "wsel": np.ascontiguousarray(wsel.astype(np.float32)),
            "wp": np.ascontiguousarray(wpc.astype(np.float32)),
            "bqk": np.ascontiguousarray(bqk), "bpr": np.ascontiguousarray(bpr),
            "eye2": eye2, "ones": np.ones((128, 130), np.float32),
        })
    return maps


def kernel(x, W_attn, b_attn, W_proj, b_proj, _trace=False, _trace_kwargs=None):
    x = np.asarray(x, np.float32)
    W_attn = np.asarray(W_attn, np.float32)
    b_attn = np.asarray(b_attn, np.float32)
    W_proj = np.asarray(W_proj, np.float32)
    b_proj = np.asarray(b_proj, np.float32)

    if "nc" not in _CACHE:
        _CACHE["nc"] = _build_program()
    nc = _CACHE["nc"]

    maps = _in_maps(x, W_attn, b_attn, W_proj, b_proj)
    kw = {}
    if _trace:
        kw = dict(trace=True, **(_trace_kwargs or {}))
    br = run_bass_kernel_spmd(nc, maps, list(range(NCORES)), **kw)
    acc = np.zeros((C, T), np.float64)
    for core in range(NCORES):
        acc += br.results[core]["outT"].astype(np.float64)
    out = np.ascontiguousarray(acc.T.astype(np.float32)).reshape(1, T, C)
    _CACHE["last_results"] = br
    return out



# revision 7
# speedup vs baseline: 1.0259x; 1.0259x over previous
"""TRN2 Bass kernel for nn_CausalSelfAttention_4054449128214.

The reference returns out_s + stop_gradient(out_full - out_s), whose forward
value is exactly out_full -- plain dense causal self-attention. So the kernel
computes: qkv = x@W_attn+b_attn, per-head causal softmax attention,
y@W_proj+b_proj.

Sharding (8 cores, no collectives): Megatron head-parallel. Cores 0-3 own head
pairs (0,1)..(6,7); cores 4-7 own heads 8..11 (duplicated for SPMD
shape-uniformity, second W_proj slice zeroed). Each core computes its heads'
QKV columns, attention, and a row-sliced partial output projection; the host
sums the 8 partials (the Megatron row-parallel all-reduce) and transposes.

v2 design (vs 46us baseline):
- everything bf16: halves DMA, enables full-rate matmuls at any free size,
  2x DVE modes. End-to-end rel err ~4e-3 (gate 2e-2).
- V is produced directly transposed ([token, channel]) by swapping matmul
  operands (stationary=x chunk, moving=W_v), killing the transpose+copy pass.
- softmax denominator via an all-ones 65th column of the V tiles (free on PE).
- causal skip: the two upper diagonal key-chunks of every 512-query tile only
  touch queries [256:512) -- scores/exp/AV run on half tiles there.
- proj uses contraction 128 (both heads at once).
- Act engine does only exp (+a few Identity copies from the same table set);
  PSUM->SBUF moves are balanced across DVE/Pool/Act by phase.
- PE p-state warmup: dummy matmul chain from t~0 so real matmuls hit the
  ramped clock.
"""

import numpy as np
import ml_dtypes

import concourse.bacc as bacc
import concourse.mybir as mybir
import concourse.tile as tile
from concourse.bass_utils import run_bass_kernel_spmd

F32 = mybir.dt.float32
BF16 = mybir.dt.bfloat16

T = 1024          # sequence length
C = 768           # channels
NH = 12           # heads
HS = 64           # head size
NCORES = 8
TT = 512          # query tile
NCC = C // 128    # 6 contraction chunks
NKC = T // 128    # 8 key chunks
SCALE = 1.0 / 8.0  # 1/sqrt(HS)

# core -> (head0, head1); cores 4-7 duplicate their head (2nd W_proj slice zeroed)
HEAD_MAP = [(0, 1), (2, 3), (4, 5), (6, 7), (8, 8), (9, 9), (10, 10), (11, 11)]

_CACHE: dict = {}


def _build_program():
    nc = bacc.Bacc("TRN2", target_bir_lowering=False, debug=False,
                   num_devices=NCORES)
    xT = nc.dram_tensor("xT", [C, T], BF16, kind="ExternalInput").ap()
    wqk = nc.dram_tensor("wqk", [C, 256], BF16, kind="ExternalInput").ap()
    wv = nc.dram_tensor("wv", [C, 128], BF16, kind="ExternalInput").ap()
    wp = nc.dram_tensor("wp", [128, C], BF16, kind="ExternalInput").ap()
    bqk = nc.dram_tensor("bqk", [128, 2], F32, kind="ExternalInput").ap()
    bpr = nc.dram_tensor("bpr", [128, NCC], F32, kind="ExternalInput").ap()
    msk = nc.dram_tensor("msk", [128, 2 * TT], BF16, kind="ExternalInput").ap()
    outT = nc.dram_tensor("outT", [C, T], BF16, kind="ExternalOutput").ap()

    with tile.TileContext(nc) as tc:
        with (
            tc.tile_pool(name="const", bufs=1) as cp,
            tc.tile_pool(name="e", bufs=6) as ep,
            tc.tile_pool(name="rb", bufs=2) as rbp,
            tc.tile_pool(name="pmm", bufs=2, space="PSUM") as pmm,
            tc.tile_pool(name="pst", bufs=3, space="PSUM") as pst,
            tc.tile_pool(name="pov", bufs=1, space="PSUM") as pov,
            tc.tile_pool(name="pvt", bufs=1, space="PSUM") as pvt,
        ):
            # ---- PE p-state warmup: keep the tensor engine busy from t~0 so
            # the clock ramps to full before the first real matmul.
            warm = cp.tile([1, 256], F32, tag="warm")
            nc.vector.memset(warm[:], 1.0)
            pwarm = pvt.tile([128, 128], F32, tag="vt")
            for _ in range(22):
                nc.tensor.matmul(pwarm[0:1, :], warm[:, 0:1], warm[:, 0:128],
                                 start=True, stop=True)

            # ---- input DMAs (SP queue). Order = need order.
            wqkt = cp.tile([128, NCC * 256], BF16, tag="wqkt")
            nc.sync.dma_start(
                out=wqkt[:].rearrange("p (c j) -> p c j", c=NCC),
                in_=wqk.rearrange("(c p) j -> p c j", p=128))
            xt = cp.tile([128, NCC * T], BF16, tag="xt")
            x3 = xT.rearrange("(c p) t -> p c t", p=128)
            xv = xt[:].rearrange("p (c t) -> p c t", c=NCC)
            for c0 in (0, 2, 4):          # first 512 tokens, cc pairs
                nc.sync.dma_start(out=xv[:, c0:c0 + 2, 0:TT],
                                  in_=x3[:, c0:c0 + 2, 0:TT])
            wvt = cp.tile([128, NCC * 128], BF16, tag="wvt")
            nc.sync.dma_start(
                out=wvt[:].rearrange("p (c j) -> p c j", c=NCC),
                in_=wv.rearrange("(c p) j -> p c j", p=128))
            for c0 in (0, 2, 4):          # second 512 tokens
                nc.sync.dma_start(out=xv[:, c0:c0 + 2, TT:T],
                                  in_=x3[:, c0:c0 + 2, TT:T])
            wpt = cp.tile([128, C], BF16, tag="wpt")
            nc.sync.dma_start(out=wpt[:], in_=wp)

            # small constants on Pool's DMA queue
            bqk_sb = cp.tile([128, 2], F32, tag="bqk")
            nc.gpsimd.dma_start(out=bqk_sb[:], in_=bqk)
            bpr_sb = cp.tile([128, NCC], F32, tag="bpr")
            nc.gpsimd.dma_start(out=bpr_sb[:], in_=bpr)
            masks = cp.tile([128, 2 * TT], BF16, tag="masks")
            nc.gpsimd.dma_start(out=masks[:], in_=msk)

            # V^T tiles: [key, 2*(64 ch + ones col)]; ones preset via memset
            vaug = [cp.tile([128, 130], BF16, tag=f"va{kc}", name=f"va{kc}")
                    for kc in range(NKC)]
            for kc in range(NKC):
                nc.gpsimd.memset(vaug[kc][:], 1.0)

            ws_q = [wqkt[:, cc * 256:cc * 256 + 128] for cc in range(NCC)]
            ws_k = [wqkt[:, cc * 256 + 128:cc * 256 + 256] for cc in range(NCC)]
            xts = [[xv[:, cc, tt * TT:(tt + 1) * TT] for tt in range(2)]
                   for cc in range(NCC)]

            qkT = [[None, None], [None, None]]   # [q/k][tt] -> [128, 512] bf16

            def emit_qkv(tt, blk, eng):
                """blk 0=q, 1=k; eng 'act'|'dve' does PSUM->SBUF (+bias)."""
                qkT[blk][tt] = cp.tile([128, TT], BF16, tag=f"qk{blk}_{tt}",
                                       name=f"qk{blk}_{tt}")
                ps = pmm.tile([128, TT], F32, tag="mm")
                w = ws_q if blk == 0 else ws_k
                for cc in range(NCC):
                    nc.tensor.matmul(ps[:], w[cc], xts[cc][tt],
                                     start=(cc == 0), stop=(cc == NCC - 1))
                if eng == "act":
                    nc.scalar.activation(
                        qkT[blk][tt][:], ps[:],
                        mybir.ActivationFunctionType.Identity,
                        bias=bqk_sb[:, blk:blk + 1])
                else:
                    nc.vector.tensor_scalar_add(
                        qkT[blk][tt][:], ps[:], bqk_sb[:, blk:blk + 1])

            def emit_vt(tc_, eng):
                """V^T for key chunk tc_ via operand-swapped matmul."""
                pv = pvt.tile([128, 128], F32, tag="vt")
                for cc in range(NCC):
                    nc.tensor.matmul(
                        pv[:], xv[:, cc, tc_ * 128:(tc_ + 1) * 128],
                        wvt[:, cc * 128:(cc + 1) * 128],
                        start=(cc == 0), stop=(cc == NCC - 1))
                dst = vaug[tc_][:, 0:130].rearrange(
                    "p (g c) -> p g c", c=65)[:, :, 0:64]
                src = pv[:].rearrange("p (g c) -> p g c", g=2)
                if eng == "act":
                    nc.scalar.activation(
                        dst, src, mybir.ActivationFunctionType.Copy)
                else:
                    nc.vector.tensor_copy(dst, src)

            yT = [None, None]     # per qt: [128, 512] bf16 (both heads)

            def attn_chunk(qt, kc):
                """One key chunk vs query tile qt, both heads."""
                kcr = kc - qt * 4
                half = kcr >= 2                # upper diagonal: queries 256:512
                qoff, qlen = (256, 256) if half else (0, TT)
                nlive = qt * 4 + 4
                pss, ees = [], []
                for hi in range(2):
                    ps = pst.tile([128, TT], F32, tag="st")
                    nc.tensor.matmul(
                        ps[:, 0:qlen],
                        qkT[1][kc // 4][hi * 64:(hi + 1) * 64,
                                        (kc % 4) * 128:(kc % 4) * 128 + 128],
                        qkT[0][qt][hi * 64:(hi + 1) * 64, qoff:qoff + qlen],
                        start=True, stop=True)
                    pss.append(ps)
                for hi in range(2):
                    e = ep.tile([128, TT], BF16, tag="e")
                    nc.scalar.activation(
                        e[:, 0:qlen], pss[hi][:, 0:qlen],
                        mybir.ActivationFunctionType.Exp, scale=SCALE)
                    if half:
                        # keep where q_local >= p + 128*(kcr-2), on Pool
                        nc.gpsimd.affine_select(
                            e[:, 0:qlen], e[:, 0:qlen], pattern=[[1, qlen]],
                            compare_op=mybir.AluOpType.is_ge, fill=0.0,
                            base=-128 * (kcr - 2), channel_multiplier=-1)
                    elif kcr >= 0:
                        m = masks[:, kcr * TT:kcr * TT + qlen]
                        nc.vector.tensor_mul(e[:, 0:qlen], e[:, 0:qlen], m)
                    ees.append(e)
                for hi in range(2):
                    nc.tensor.matmul(
                        po[hi][:, qoff:qoff + qlen],
                        vaug[kc][:, hi * 65:(hi + 1) * 65], ees[hi][:, 0:qlen],
                        start=(kc == 0), stop=(kc == nlive - 1),
                        skip_group_check=half)

            def emit_norm(qt):
                """softmax divide; fills yT[qt] rows hi*64:(hi+1)*64."""
                yT[qt] = cp.tile([128, TT], BF16, tag=f"y{qt}", name=f"y{qt}")
                for hi in range(2):
                    rb = rbp.tile([1, TT], F32, tag="rb")
                    nc.vector.reciprocal(rb[:], po[hi][64:65, :])
                    rbc = rbp.tile([64, TT], F32, tag="rbc")
                    nc.gpsimd.partition_broadcast(rbc[:], rb[0:1, :])
                    nc.vector.tensor_mul(
                        yT[qt][hi * 64:(hi + 1) * 64, :], po[hi][0:64, :],
                        rbc[:])

            ost = [cp.tile([128, 3 * TT], BF16, tag=f"ost{i}", name=f"ost{i}")
                   for i in range(4)]     # staging: (tt, et-half) -> 3 ets

            def emit_proj_et(tt, et, eng):
                pm = pmm.tile([128, TT], F32, tag="mm")
                nc.tensor.matmul(pm[:], wpt[:, et * 128:(et + 1) * 128],
                                 yT[tt][:], start=True, stop=True)
                st = ost[tt * 2 + et // 3]
                dst = st[:, (et % 3) * TT:(et % 3) * TT + TT]
                if eng == "pool":
                    nc.gpsimd.tensor_scalar_add(dst, pm[:], bpr_sb[:, et:et + 1])
                elif eng == "dve":
                    nc.vector.tensor_scalar_add(dst, pm[:], bpr_sb[:, et:et + 1])
                else:
                    nc.scalar.activation(
                        dst, pm[:], mybir.ActivationFunctionType.Identity,
                        bias=bpr_sb[:, et:et + 1])

            def emit_out_dma(tt, half):
                st = ost[tt * 2 + half]
                nc.sync.dma_start(
                    out=outT[half * 384:half * 384 + 384,
                             tt * TT:(tt + 1) * TT]
                    .rearrange("(g p) t -> p g t", p=128),
                    in_=st[:].rearrange("p (g t) -> p g t", g=3))

            # ================= schedule =================
            emit_qkv(0, 0, "act")
            emit_qkv(0, 1, "act")
            for tc_ in range(4):
                emit_vt(tc_, "act")

            po = [pov.tile([65, TT], F32, tag=f"po{hi}", name=f"po{hi}_a")
                  for hi in range(2)]
            attn_chunk(0, 0)
            attn_chunk(0, 1)
            emit_qkv(1, 0, "dve")
            attn_chunk(0, 2)
            emit_qkv(1, 1, "dve")
            attn_chunk(0, 3)
            po0 = po
            emit_norm(0)

            po = [pov.tile([65, TT], F32, tag=f"po{hi}", name=f"po{hi}_b")
                  for hi in range(2)]
            attn_chunk(1, 0)
            emit_vt(4, "dve")
            attn_chunk(1, 1)
            emit_vt(5, "dve")
            attn_chunk(1, 2)
            emit_vt(6, "dve")
            attn_chunk(1, 3)
            emit_vt(7, "dve")
            attn_chunk(1, 4)
            emit_proj_et(0, 0, "dve")
            emit_proj_et(0, 1, "dve")
            attn_chunk(1, 5)
            emit_proj_et(0, 2, "dve")
            emit_proj_et(0, 3, "dve")
            attn_chunk(1, 6)
            emit_out_dma(0, 0)
            emit_proj_et(0, 4, "dve")
            emit_proj_et(0, 5, "dve")
            attn_chunk(1, 7)
            emit_out_dma(0, 1)
            emit_norm(1)
            emit_proj_et(1, 0, "dve")
            emit_proj_et(1, 1, "act")
            emit_proj_et(1, 2, "dve")
            emit_out_dma(1, 0)
            emit_proj_et(1, 3, "act")
            emit_proj_et(1, 4, "dve")
            emit_proj_et(1, 5, "act")
            emit_out_dma(1, 1)
    nc.compile()
    return nc


def _bf16(a):
    return np.ascontiguousarray(np.asarray(a, np.float32)).astype(
        ml_dtypes.bfloat16)


def _in_maps(x, W_attn, b_attn, W_proj, b_proj):
    xTn = _bf16(x.reshape(T, C).T)                       # [C, T]
    # causal masks for the two diagonal chunk offsets (0 and 128)
    p = np.arange(128)[:, None]
    q = np.arange(TT)[None, :]
    msk = np.concatenate(
        [(q >= p).astype(np.float32), (q >= p + 128).astype(np.float32)],
        axis=1)                                          # [128, 1024]
    mskb = _bf16(msk)
    b_eff = (b_proj + b_attn[2 * C:] @ W_proj).astype(np.float32)
    maps = []
    for core in range(NCORES):
        h0, h1 = HEAD_MAP[core]
        qc = list(range(h0 * HS, (h0 + 1) * HS)) + \
            list(range(h1 * HS, (h1 + 1) * HS))
        wqk = np.concatenate(
            [W_attn[:, qc], W_attn[:, [C + i for i in qc]]], axis=1)
        wvc = W_attn[:, [2 * C + i for i in qc]]
        wpc = np.concatenate(
            [W_proj[h0 * HS:(h0 + 1) * HS, :],
             np.zeros_like(W_proj[:HS]) if h1 == h0
             else W_proj[h1 * HS:(h1 + 1) * HS, :]], axis=0)   # [128, C]
        bqkc = np.stack([np.concatenate([b_attn[p * C + h0 * HS:
                                                p * C + (h0 + 1) * HS],
                                         b_attn[p * C + h1 * HS:
                                                p * C + (h1 + 1) * HS]])
                         for p in range(2)], axis=1).astype(np.float32)
        bprc = (b_eff.reshape(NCC, 128).T if core == 0
                else np.zeros((128, NCC), np.float32))
        maps.append({
            "xT": xTn, "wqk": _bf16(wqk), "wv": _bf16(wvc),
            "wp": _bf16(wpc), "bqk": np.ascontiguousarray(bqkc),
            "bpr": np.ascontiguousarray(bprc), "msk": mskb,
        })
    return maps


def kernel(x, W_attn, b_attn, W_proj, b_proj, _trace=False, _trace_kwargs=None):
    x = np.asarray(x, np.float32)
    W_attn = np.asarray(W_attn, np.float32)
    b_attn = np.asarray(b_attn, np.float32)
    W_proj = np.asarray(W_proj, np.float32)
    b_proj = np.asarray(b_proj, np.float32)

    if "nc" not in _CACHE:
        _CACHE["nc"] = _build_program()
    nc = _CACHE["nc"]

    maps = _in_maps(x, W_attn, b_attn, W_proj, b_proj)
    kw = {}
    if _trace:
        kw = dict(trace=True, **(_trace_kwargs or {}))
    br = run_bass_kernel_spmd(nc, maps, list(range(NCORES)), **kw)
    acc = np.zeros((C, T), np.float64)
    for core in range(NCORES):
        acc += br.results[core]["outT"].astype(np.float64)
    out = np.ascontiguousarray(acc.T.astype(np.float32)).reshape(1, T, C)
    _CACHE["last_results"] = br
    return out


# revision 8
# speedup vs baseline: 1.1036x; 1.0758x over previous
"""TRN2 Bass kernel for nn_CausalSelfAttention_4054449128214.

The reference returns out_s + stop_gradient(out_full - out_s), whose forward
value is exactly out_full -- plain dense causal self-attention. So the kernel
computes: qkv = x@W_attn+b_attn, per-head causal softmax attention,
y@W_proj+b_proj.

Sharding (8 cores, no collectives): Megatron head-parallel. Cores 0-3 own head
pairs (0,1)..(6,7); cores 4-7 own heads 8..11 (duplicated for SPMD
shape-uniformity, second W_proj slice zeroed). Each core computes its heads'
QKV columns, attention, and a row-sliced partial output projection; the host
sums the 8 partials (the Megatron row-parallel all-reduce) and transposes.

v2 design (vs 46us baseline):
- everything bf16: halves DMA, enables full-rate matmuls at any free size,
  2x DVE modes. End-to-end rel err ~4e-3 (gate 2e-2).
- V is produced directly transposed ([token, channel]) by swapping matmul
  operands (stationary=x chunk, moving=W_v), killing the transpose+copy pass.
- softmax denominator via an all-ones 65th column of the V tiles (free on PE).
- causal skip: the two upper diagonal key-chunks of every 512-query tile only
  touch queries [256:512) -- scores/exp/AV run on half tiles there.
- proj uses contraction 128 (both heads at once).
- Act engine does only exp (+a few Identity copies from the same table set);
  PSUM->SBUF moves are balanced across DVE/Pool/Act by phase.
- PE p-state warmup: dummy matmul chain from t~0 so real matmuls hit the
  ramped clock.
"""

import numpy as np
import ml_dtypes

import concourse.bacc as bacc
import concourse.mybir as mybir
import concourse.tile as tile
from concourse.bass_utils import run_bass_kernel_spmd

F32 = mybir.dt.float32
BF16 = mybir.dt.bfloat16

T = 1024          # sequence length
C = 768           # channels
NH = 12           # heads
HS = 64           # head size
NCORES = 8
TT = 512          # query tile
NCC = C // 128    # 6 contraction chunks
NKC = T // 128    # 8 key chunks
SCALE = 1.0 / 8.0  # 1/sqrt(HS)

# core -> (head0, head1); cores 4-7 duplicate their head (2nd W_proj slice zeroed)
HEAD_MAP = [(0, 1), (2, 3), (4, 5), (6, 7), (8, 8), (9, 9), (10, 10), (11, 11)]

_CACHE: dict = {}


def _build_program():
    nc = bacc.Bacc("TRN2", target_bir_lowering=False, debug=False,
                   num_devices=NCORES)
    xT = nc.dram_tensor("xT", [C, T], BF16, kind="ExternalInput").ap()
    wqk = nc.dram_tensor("wqk", [C, 256], BF16, kind="ExternalInput").ap()
    wv = nc.dram_tensor("wv", [C, 128], BF16, kind="ExternalInput").ap()
    wp = nc.dram_tensor("wp", [128, C], BF16, kind="ExternalInput").ap()
    bqk = nc.dram_tensor("bqk", [128, 2], F32, kind="ExternalInput").ap()
    bpr = nc.dram_tensor("bpr", [128, NCC], F32, kind="ExternalInput").ap()
    msk = nc.dram_tensor("msk", [128, 2 * TT], BF16, kind="ExternalInput").ap()
    outT = nc.dram_tensor("outT", [C, T], BF16, kind="ExternalOutput").ap()

    with tile.TileContext(nc) as tc:
        with (
            tc.tile_pool(name="const", bufs=1) as cp,
            tc.tile_pool(name="e", bufs=6) as ep,
            tc.tile_pool(name="rb", bufs=2) as rbp,
            tc.tile_pool(name="pmm", bufs=2, space="PSUM") as pmm,
            tc.tile_pool(name="pst", bufs=3, space="PSUM") as pst,
            tc.tile_pool(name="pov", bufs=1, space="PSUM") as pov,
            tc.tile_pool(name="pvt", bufs=1, space="PSUM") as pvt,
        ):
            # ---- input DMAs (SP queue). Order = need order.
            wqkt = cp.tile([128, NCC * 256], BF16, tag="wqkt")
            nc.sync.dma_start(
                out=wqkt[:].rearrange("p (c j) -> p c j", c=NCC),
                in_=wqk.rearrange("(c p) j -> p c j", p=128))
            xt = cp.tile([128, NCC * T], BF16, tag="xt")
            x3 = xT.rearrange("(c p) t -> p c t", p=128)
            xv = xt[:].rearrange("p (c t) -> p c t", c=NCC)
            for c0 in (0, 2, 4):          # first 512 tokens, cc pairs
                nc.sync.dma_start(out=xv[:, c0:c0 + 2, 0:TT],
                                  in_=x3[:, c0:c0 + 2, 0:TT])
            wvt = cp.tile([128, NCC * 128], BF16, tag="wvt")
            nc.sync.dma_start(
                out=wvt[:].rearrange("p (c j) -> p c j", c=NCC),
                in_=wv.rearrange("(c p) j -> p c j", p=128))
            for c0 in (0, 2, 4):          # second 512 tokens
                nc.sync.dma_start(out=xv[:, c0:c0 + 2, TT:T],
                                  in_=x3[:, c0:c0 + 2, TT:T])
            wpt = cp.tile([128, C], BF16, tag="wpt")
            nc.sync.dma_start(out=wpt[:], in_=wp)

            # small constants on Pool's DMA queue
            bqk_sb = cp.tile([128, 2], F32, tag="bqk")
            nc.gpsimd.dma_start(out=bqk_sb[:], in_=bqk)
            bpr_sb = cp.tile([128, NCC], F32, tag="bpr")
            nc.gpsimd.dma_start(out=bpr_sb[:], in_=bpr)
            masks = cp.tile([128, 2 * TT], BF16, tag="masks")
            nc.gpsimd.dma_start(out=masks[:], in_=msk)

            # V^T tiles: [key, 2*(64 ch + ones col)]; ones preset via memset
            vaug = [cp.tile([128, 130], BF16, tag=f"va{kc}", name=f"va{kc}")
                    for kc in range(NKC)]
            for kc in range(NKC):
                nc.gpsimd.memset(vaug[kc][:], 1.0)

            ws_q = [wqkt[:, cc * 256:cc * 256 + 128] for cc in range(NCC)]
            ws_k = [wqkt[:, cc * 256 + 128:cc * 256 + 256] for cc in range(NCC)]
            xts = [[xv[:, cc, tt * TT:(tt + 1) * TT] for tt in range(2)]
                   for cc in range(NCC)]

            qkT = [[None, None], [None, None]]   # [q/k][tt] -> [128, 512] bf16

            def emit_qkv(tt, blk, eng):
                """blk 0=q, 1=k; eng 'act'|'dve' does PSUM->SBUF (+bias)."""
                qkT[blk][tt] = cp.tile([128, TT], BF16, tag=f"qk{blk}_{tt}",
                                       name=f"qk{blk}_{tt}")
                ps = pmm.tile([128, TT], F32, tag="mm")
                w = ws_q if blk == 0 else ws_k
                for cc in range(NCC):
                    nc.tensor.matmul(ps[:], w[cc], xts[cc][tt],
                                     start=(cc == 0), stop=(cc == NCC - 1))
                if eng == "act":
                    nc.scalar.activation(
                        qkT[blk][tt][:], ps[:],
                        mybir.ActivationFunctionType.Identity,
                        bias=bqk_sb[:, blk:blk + 1])
                else:
                    nc.vector.tensor_scalar_add(
                        qkT[blk][tt][:], ps[:], bqk_sb[:, blk:blk + 1])

            def emit_vt(tc_, eng):
                """V^T for key chunk tc_ via operand-swapped matmul."""
                pv = pvt.tile([128, 128], F32, tag="vt")
                for cc in range(NCC):
                    nc.tensor.matmul(
                        pv[:], xv[:, cc, tc_ * 128:(tc_ + 1) * 128],
                        wvt[:, cc * 128:(cc + 1) * 128],
                        start=(cc == 0), stop=(cc == NCC - 1))
                dst = vaug[tc_][:, 0:130].rearrange(
                    "p (g c) -> p g c", c=65)[:, :, 0:64]
                src = pv[:].rearrange("p (g c) -> p g c", g=2)
                if eng == "act":
                    nc.scalar.activation(
                        dst, src, mybir.ActivationFunctionType.Copy)
                else:
                    nc.vector.tensor_copy(dst, src)

            yT = [None, None]     # per qt: [128, 512] bf16 (both heads)

            def attn_chunk(qt, kc):
                """One key chunk vs query tile qt, both heads."""
                kcr = kc - qt * 4
                half = kcr >= 2                # upper diagonal: queries 256:512
                qoff, qlen = (256, 256) if half else (0, TT)
                nlive = qt * 4 + 4
                pss, ees = [], []
                for hi in range(2):
                    ps = pst.tile([128, TT], F32, tag="st")
                    nc.tensor.matmul(
                        ps[:, 0:qlen],
                        qkT[1][kc // 4][hi * 64:(hi + 1) * 64,
                                        (kc % 4) * 128:(kc % 4) * 128 + 128],
                        qkT[0][qt][hi * 64:(hi + 1) * 64, qoff:qoff + qlen],
                        start=True, stop=True)
                    pss.append(ps)
                for hi in range(2):
                    e = ep.tile([128, TT], BF16, tag="e")
                    nc.scalar.activation(
                        e[:, 0:qlen], pss[hi][:, 0:qlen],
                        mybir.ActivationFunctionType.Exp, scale=SCALE)
                    if half:
                        # keep where q_local >= p + 128*(kcr-2), on Pool
                        nc.gpsimd.affine_select(
                            e[:, 0:qlen], e[:, 0:qlen], pattern=[[1, qlen]],
                            compare_op=mybir.AluOpType.is_ge, fill=0.0,
                            base=-128 * (kcr - 2), channel_multiplier=-1)
                    elif kcr >= 0:
                        m = masks[:, kcr * TT:kcr * TT + qlen]
                        nc.vector.tensor_mul(e[:, 0:qlen], e[:, 0:qlen], m)
                    ees.append(e)
                for hi in range(2):
                    nc.tensor.matmul(
                        po[hi][:, qoff:qoff + qlen],
                        vaug[kc][:, hi * 65:(hi + 1) * 65], ees[hi][:, 0:qlen],
                        start=(kc == 0), stop=(kc == nlive - 1),
                        skip_group_check=half)

            def emit_norm(qt):
                """softmax divide; fills yT[qt] rows hi*64:(hi+1)*64."""
                yT[qt] = cp.tile([128, TT], BF16, tag=f"y{qt}", name=f"y{qt}")
                for hi in range(2):
                    rb = rbp.tile([1, TT], F32, tag="rb")
                    nc.vector.reciprocal(rb[:], po[hi][64:65, :])
                    rbc = rbp.tile([64, TT], F32, tag="rbc")
                    nc.gpsimd.partition_broadcast(rbc[:], rb[0:1, :])
                    nc.vector.tensor_mul(
                        yT[qt][hi * 64:(hi + 1) * 64, :], po[hi][0:64, :],
                        rbc[:])

            ost = [cp.tile([128, 3 * TT], BF16, tag=f"ost{i}", name=f"ost{i}")
                   for i in range(4)]     # staging: (tt, et-half) -> 3 ets

            def emit_proj_et(tt, et, eng):
                pm = pmm.tile([128, TT], F32, tag="mm")
                nc.tensor.matmul(pm[:], wpt[:, et * 128:(et + 1) * 128],
                                 yT[tt][:], start=True, stop=True)
                st = ost[tt * 2 + et // 3]
                dst = st[:, (et % 3) * TT:(et % 3) * TT + TT]
                if eng == "pool":
                    nc.gpsimd.tensor_scalar_add(dst, pm[:], bpr_sb[:, et:et + 1])
                elif eng == "dve":
                    nc.vector.tensor_scalar_add(dst, pm[:], bpr_sb[:, et:et + 1])
                else:
                    nc.scalar.activation(
                        dst, pm[:], mybir.ActivationFunctionType.Identity,
                        bias=bpr_sb[:, et:et + 1])

            def emit_out_dma(tt, half):
                st = ost[tt * 2 + half]
                nc.sync.dma_start(
                    out=outT[half * 384:half * 384 + 384,
                             tt * TT:(tt + 1) * TT]
                    .rearrange("(g p) t -> p g t", p=128),
                    in_=st[:].rearrange("p (g t) -> p g t", g=3))

            # ================= schedule =================
            emit_qkv(0, 0, "act")
            emit_qkv(0, 1, "act")
            for tc_ in range(4):
                emit_vt(tc_, "act")

            po = [pov.tile([65, TT], F32, tag=f"po{hi}", name=f"po{hi}_a")
                  for hi in range(2)]
            attn_chunk(0, 0)
            attn_chunk(0, 1)
            emit_qkv(1, 0, "dve")
            attn_chunk(0, 2)
            emit_qkv(1, 1, "dve")
            attn_chunk(0, 3)
            po0 = po
            emit_norm(0)

            po = [pov.tile([65, TT], F32, tag=f"po{hi}", name=f"po{hi}_b")
                  for hi in range(2)]
            attn_chunk(1, 0)
            emit_vt(4, "dve")
            attn_chunk(1, 1)
            emit_vt(5, "dve")
            attn_chunk(1, 2)
            emit_vt(6, "dve")
            attn_chunk(1, 3)
            emit_vt(7, "dve")
            attn_chunk(1, 4)
            emit_proj_et(0, 0, "dve")
            emit_proj_et(0, 1, "dve")
            attn_chunk(1, 5)
            emit_proj_et(0, 2, "dve")
            emit_proj_et(0, 3, "dve")
            attn_chunk(1, 6)
            emit_out_dma(0, 0)
            emit_proj_et(0, 4, "dve")
            emit_proj_et(0, 5, "dve")
            attn_chunk(1, 7)
            emit_out_dma(0, 1)
            emit_norm(1)
            emit_proj_et(1, 0, "dve")
            emit_proj_et(1, 1, "act")
            emit_proj_et(1, 2, "dve")
            emit_out_dma(1, 0)
            emit_proj_et(1, 3, "act")
            emit_proj_et(1, 4, "dve")
            emit_proj_et(1, 5, "act")
            emit_out_dma(1, 1)
    nc.compile()
    return nc


def _bf16(a):
    return np.ascontiguousarray(np.asarray(a, np.float32)).astype(
        ml_dtypes.bfloat16)


def _in_maps(x, W_attn, b_attn, W_proj, b_proj):
    xTn = _bf16(x.reshape(T, C).T)                       # [C, T]
    # causal masks for the two diagonal chunk offsets (0 and 128)
    p = np.arange(128)[:, None]
    q = np.arange(TT)[None, :]
    msk = np.concatenate(
        [(q >= p).astype(np.float32), (q >= p + 128).astype(np.float32)],
        axis=1)                                          # [128, 1024]
    mskb = _bf16(msk)
    b_eff = (b_proj + b_attn[2 * C:] @ W_proj).astype(np.float32)
    maps = []
    for core in range(NCORES):
        h0, h1 = HEAD_MAP[core]
        qc = list(range(h0 * HS, (h0 + 1) * HS)) + \
            list(range(h1 * HS, (h1 + 1) * HS))
        wqk = np.concatenate(
            [W_attn[:, qc], W_attn[:, [C + i for i in qc]]], axis=1)
        wvc = W_attn[:, [2 * C + i for i in qc]]
        wpc = np.concatenate(
            [W_proj[h0 * HS:(h0 + 1) * HS, :],
             np.zeros_like(W_proj[:HS]) if h1 == h0
             else W_proj[h1 * HS:(h1 + 1) * HS, :]], axis=0)   # [128, C]
        bqkc = np.stack([np.concatenate([b_attn[p * C + h0 * HS:
                                                p * C + (h0 + 1) * HS],
                                         b_attn[p * C + h1 * HS:
                                                p * C + (h1 + 1) * HS]])
                         for p in range(2)], axis=1).astype(np.float32)
        bprc = (b_eff.reshape(NCC, 128).T if core == 0
                else np.zeros((128, NCC), np.float32))
        maps.append({
            "xT": xTn, "wqk": _bf16(wqk), "wv": _bf16(wvc),
            "wp": _bf16(wpc), "bqk": np.ascontiguousarray(bqkc),
            "bpr": np.ascontiguousarray(bprc), "msk": mskb,
        })
    return maps


def kernel(x, W_attn, b_attn, W_proj, b_proj, _trace=False, _trace_kwargs=None):
    x = np.asarray(x, np.float32)
    W_attn = np.asarray(W_attn, np.float32)
    b_attn = np.asarray(b_attn, np.float32)
    W_proj = np.asarray(W_proj, np.float32)
    b_proj = np.asarray(b_proj, np.float32)

    if "nc" not in _CACHE:
        _CACHE["nc"] = _build_program()
    nc = _CACHE["nc"]

    maps = _in_maps(x, W_attn, b_attn, W_proj, b_proj)
    kw = {}
    if _trace:
        kw = dict(trace=True, **(_trace_kwargs or {}))
    br = run_bass_kernel_spmd(nc, maps, list(range(NCORES)), **kw)
    acc = np.zeros((C, T), np.float64)
    for core in range(NCORES):
        acc += br.results[core]["outT"].astype(np.float64)
    out = np.ascontiguousarray(acc.T.astype(np.float32)).reshape(1, T, C)
    _CACHE["last_results"] = br
    return out
